# revision 8
# baseline (speedup 1.0000x reference)
"""Trainium2 Bass kernel for nn_MemoryRetriever (retrieval_knn).

Strategy
--------
- 16384 tokens (B*S) sharded as 2048 tokens/core across 8 cores; all weights
  replicated (bf16, pre-transposed on host).
- Top-k memory selection: softmax over the selected memories is permutation
  invariant, so only the top-k *set* matters -> computed on host (argsort of
  4096 scalars), along with the tiny mem gather + LayerNorm + k/v projections
  (512x256 rows, ~0.03% of total FLOPs).
- Device kernel per core: 4 token tiles of 512. Matmuls in bf16 (fp32 PSUM
  accumulation):
    * q-path + attention in "feature-major" (transposed activations): no
      on-device transposes needed; scores computed directly as S^T [kk, t];
      softmax denominators ride along the ctx matmul as an extra ones-column
      of v; no max-subtraction (|scores/8| < 1 for this distribution).
    * MLP (gate / int1 / int2) token-major: LayerNorm stats via ACT accum_out
      during PSUM eviction; (x-mu)*rstd fused into the gelu activation's
      scale/bias; sigmoid computed as tanh to keep the ACT table-set switches
      to 2 per tile (exp/ln home set + gelu/tanh set).
    * gelu input (mid) transposed back via PE transposes (bf16) for int2.
- X loaded twice: bf16 transposed via DMA-transpose (matmul operand), f32
  natural (final residual + output LayerNorm in fp32).
"""

import sys
import os

for _p in ("/opt/trn_rl_repo", os.path.expanduser("~/.axon_site/_ro/trn_rl_repo")):
    if os.path.isdir(_p) and _p not in sys.path:
        sys.path.insert(0, _p)

import numpy as np
import ml_dtypes
from contextlib import ExitStack

from concourse import bass, bacc, tile, mybir
from concourse.bass_utils import run_bass_kernel_spmd
from concourse.masks import make_identity

bf16 = mybir.dt.bfloat16
f32 = mybir.dt.float32
AF = mybir.ActivationFunctionType
ALU = mybir.AluOpType
AX = mybir.AxisListType
BF = ml_dtypes.bfloat16

H = 1024      # hidden size
M = 256       # memory dim
NH = 4        # heads
HD = 64       # head dim
KSEL = 512    # top_k
NCORES = 8
TPC = 2048    # tokens per core
TT = 512      # tokens per tile
NTILES = TPC // TT
EPS = 1e-5

HC = H // 128        # 8 chunks of hidden
CC = (H + M) // 128  # 10 chunks of cat dim (1280)
MC = M // 128        # 2 chunks of memory dim
IC = 2 * H // 128    # 16 chunks of intermediate (2048)
KC = KSEL // 128     # 4 chunks of selected memories


def _emit(tc, nc, flags):
    """Emit the per-core program. flags: dict of conditional-path switches."""
    ctx = tc._kernel_ctx  # ExitStack owned by caller

    # ---- DRAM tensors -------------------------------------------------
    xq = nc.dram_tensor("xq", [TPC, H], f32, kind="ExternalInput").ap()
    xbf = nc.dram_tensor("xbf", [TPC, HC, 128], bf16, kind="ExternalInput").ap()
    qp_wT = nc.dram_tensor("qp_wT", [128, HC, M], bf16, kind="ExternalInput").ap()
    qp_b = nc.dram_tensor("qp_b", [128, MC, 1], f32, kind="ExternalInput").ap()
    wqT = nc.dram_tensor("wqT", [128, MC, M], bf16, kind="ExternalInput").ap()
    bq = nc.dram_tensor("bq", [128, MC, 1], f32, kind="ExternalInput").ap()
    kTd = nc.dram_tensor("kT", [128, MC, KSEL], bf16, kind="ExternalInput").ap()
    vaug = nc.dram_tensor("vaug", [128, KC, NH, HD + 1], bf16, kind="ExternalInput").ap()
    out_wT = nc.dram_tensor("out_wT", [128, MC, M], bf16, kind="ExternalInput").ap()
    out_b = nc.dram_tensor("out_b", [128, MC, 1], f32, kind="ExternalInput").ap()
    gate_wT = nc.dram_tensor("gate_wT", [128, CC, H], bf16, kind="ExternalInput").ap()
    w1T = nc.dram_tensor("w1T", [128, CC, 2 * H], bf16, kind="ExternalInput").ap()
    w2T = nc.dram_tensor("w2T", [128, IC, H], bf16, kind="ExternalInput").ap()
    out_d = nc.dram_tensor("out", [TPC, H], f32, kind="ExternalOutput").ap()

    # optional general-path tensors (replicated across partitions on host)
    opt = {}
    if flags["gate_b"]:
        opt["gate_b_rep"] = nc.dram_tensor("gate_b_rep", [128, H], f32, kind="ExternalInput").ap()
    if flags["int_b1"]:
        opt["b1_rep"] = nc.dram_tensor("b1_rep", [128, 2 * H], f32, kind="ExternalInput").ap()
    if flags["int_b2"]:
        opt["b2h_rep"] = nc.dram_tensor("b2h_rep", [128, H], f32, kind="ExternalInput").ap()
    if flags["int_ln"]:
        opt["g1_rep"] = nc.dram_tensor("g1_rep", [128, 2 * H], f32, kind="ExternalInput").ap()
        opt["b1ln_rep"] = nc.dram_tensor("b1ln_rep", [128, 2 * H], f32, kind="ExternalInput").ap()
    if flags["ln2"]:
        opt["g2_rep"] = nc.dram_tensor("g2_rep", [128, H], f32, kind="ExternalInput").ap()
        opt["b2ln_rep"] = nc.dram_tensor("b2ln_rep", [128, H], f32, kind="ExternalInput").ap()

    # ---- pools --------------------------------------------------------
    singles = ctx.enter_context(tc.tile_pool(name="singles", bufs=1))
    cat_p = ctx.enter_context(tc.tile_pool(name="cat", bufs=2))
    x_p = ctx.enter_context(tc.tile_pool(name="xp", bufs=2))
    qe_p = ctx.enter_context(tc.tile_pool(name="qe", bufs=2))
    pt_p = ctx.enter_context(tc.tile_pool(name="pt", bufs=3))
    ctx_p = ctx.enter_context(tc.tile_pool(name="ctxp", bufs=2))
    rc_p = ctx.enter_context(tc.tile_pool(name="rc", bufs=2))
    bc_p = ctx.enter_context(tc.tile_pool(name="bc", bufs=2))
    th_p = ctx.enter_context(tc.tile_pool(name="th", bufs=2))
    mid_p = ctx.enter_context(tc.tile_pool(name="mid", bufs=2))
    midg_p = ctx.enter_context(tc.tile_pool(name="midg", bufs=2))
    sq_p = ctx.enter_context(tc.tile_pool(name="sq", bufs=1))
    mgt_p = ctx.enter_context(tc.tile_pool(name="mgt", bufs=3))
    res_p = ctx.enter_context(tc.tile_pool(name="res", bufs=2))
    st_p = ctx.enter_context(tc.tile_pool(name="st", bufs=4))
    tmp_p = ctx.enter_context(tc.tile_pool(name="tmp", bufs=2))

    ps_p = ctx.enter_context(tc.tile_pool(name="ps", bufs=4, space="PSUM"))
    ctxps_p = ctx.enter_context(tc.tile_pool(name="ctxps", bufs=2, space="PSUM"))
    tp_p = ctx.enter_context(tc.tile_pool(name="tp", bufs=2, space="PSUM"))

    # ---- load weights -------------------------------------------------
    def load(tag, ap, shape, dtype):
        t = singles.tile(shape, dtype, tag=tag, name=f"w_{tag}")
        nc.sync.dma_start(t[:], ap[:])
        return t

    qp_wT_s = load("qp_wT", qp_wT, [128, HC, M], bf16)
    qp_b_s = load("qp_b", qp_b, [128, MC, 1], f32)
    wqT_s = load("wqT", wqT, [128, MC, M], bf16)
    bq_s = load("bq", bq, [128, MC, 1], f32)
    kT_s = load("kT", kTd, [128, MC, KSEL], bf16)
    vaug_s = load("vaug", vaug, [128, KC, NH, HD + 1], bf16)
    out_wT_s = load("out_wT", out_wT, [128, MC, M], bf16)
    out_b_s = load("out_b", out_b, [128, MC, 1], f32)
    gate_wT_s = load("gate_wT", gate_wT, [128, CC, H], bf16)
    w1T_s = load("w1T", w1T, [128, CC, 2 * H], bf16)
    w2T_s = load("w2T", w2T, [128, IC, H], bf16)
    opt_s = {k: load(k, v, list(v.shape), v.dtype) for k, v in opt.items()}

    ident = singles.tile([128, 128], bf16)
    make_identity(nc, ident[:])
    eps_s = singles.tile([128, 1], f32)
    nc.vector.memset(eps_s[:], EPS)

    # ---- per-tile program --------------------------------------------
    for t in range(NTILES):
        trow = t * TT

        # cat buffer: chunks 0..7 = X^T (bf16), 8..9 = attn_out^T
        cat = cat_p.tile([128, CC, TT], bf16)
        nc.sync.dma_start_transpose(cat[:, 0:HC, :], xbf[trow:trow + TT, :, :])

        # ---- q projection: q_embT[m, t] (feature-major) ----
        qe = qe_p.tile([128, MC, TT], bf16, tag="qe")
        for j in range(MC):
            ps = ps_p.tile([128, TT], f32, tag="ps")
            for c in range(HC):
                nc.tensor.matmul(ps[:], qp_wT_s[:, c, 128 * j:128 * (j + 1)],
                                 cat[:, c, :], start=(c == 0), stop=(c == HC - 1))
            nc.scalar.activation(qe[:, j, :], ps[:], AF.Identity, bias=qp_b_s[:, j, :])

        qT = qe_p.tile([128, MC, TT], bf16, tag="qT")
        for j in range(MC):
            ps = ps_p.tile([128, TT], f32, tag="ps")
            for c in range(MC):
                nc.tensor.matmul(ps[:], wqT_s[:, c, 128 * j:128 * (j + 1)],
                                 qe[:, c, :], start=(c == 0), stop=(c == MC - 1))
            nc.scalar.activation(qT[:, j, :], ps[:], AF.Identity, bias=bq_s[:, j, :])

        # ---- attention, per head ----
        ctxT = ctx_p.tile([128, MC, TT], bf16)
        for h in range(NH):
            hi, off = h // 2, 64 * (h % 2)
            # P^T = exp(scores^T / 8), kk-major; denominator rides as v column 64
            cps = ctxps_p.tile([65, TT], f32, tag="cps")
            for kc in range(KC):
                sps = ps_p.tile([128, TT], f32, tag="ps")
                nc.tensor.matmul(sps[:], kT_s[off:off + 64, hi, 128 * kc:128 * (kc + 1)],
                                 qT[off:off + 64, hi, :], start=True, stop=True)
                pt = pt_p.tile([128, TT], bf16, tag="pt")
                nc.scalar.activation(pt[:], sps[:], AF.Exp, scale=0.125)
                nc.tensor.matmul(cps[:], vaug_s[:, kc, h, :], pt[:],
                                 start=(kc == 0), stop=(kc == KC - 1))
            rec = rc_p.tile([1, TT], f32, tag="rec")
            nc.vector.reciprocal(rec[:], cps[64:65, :])
            bca = bc_p.tile([64, TT], f32, tag="bca")
            nc.gpsimd.partition_broadcast(bca[:], rec[:])
            nc.vector.tensor_tensor(out=ctxT[off:off + 64, hi, :], in0=cps[0:64, :],
                                    in1=bca[:], op=ALU.mult)

        # ---- attention output projection -> cat chunks 8,9 ----
        for j in range(MC):
            ps = ps_p.tile([128, TT], f32, tag="ps")
            for c in range(MC):
                nc.tensor.matmul(ps[:], out_wT_s[:, c, 128 * j:128 * (j + 1)],
                                 ctxT[:, c, :], start=(c == 0), stop=(c == MC - 1))
            nc.scalar.activation(cat[:, HC + j, :], ps[:], AF.Identity,
                                 bias=out_b_s[:, j, :])

        # ---- MLP, fully per 128-token chunk ----
        for tch in range(TT // 128):
            tsl = slice(128 * tch, 128 * (tch + 1))

            xn = x_p.tile([128, H], f32, tag="xn")
            nc.sync.dma_start(xn[:], xq[trow + tch * 128: trow + (tch + 1) * 128, :])

            # int1 + LN stats (sums via ACT accum during PSUM eviction)
            mid = mid_p.tile([128, 2 * H], bf16, tag="mid")
            sti = st_p.tile([128, 16], f32, tag="sti")
            for n in range(4):
                ps = ps_p.tile([128, 512], f32, tag="ps")
                for c in range(CC):
                    nc.tensor.matmul(ps[:], cat[:, c, tsl], w1T_s[:, c, 512 * n:512 * (n + 1)],
                                     start=(c == 0), stop=(c == CC - 1))
                nsl = slice(512 * n, 512 * (n + 1))
                if flags["int_b1"]:
                    nc.vector.tensor_tensor(out=mid[:, nsl], in0=ps[:],
                                            in1=opt_s["b1_rep"][:, nsl], op=ALU.add)
                    sqs = sq_p.tile([128, 512], bf16, tag="sqs")
                    nc.scalar.activation(sqs[:], mid[:, nsl], AF.Identity,
                                         accum_out=sti[:, n:n + 1])
                    nc.scalar.activation(sqs[:], mid[:, nsl], AF.Square,
                                         accum_out=sti[:, 4 + n:5 + n])
                else:
                    nc.scalar.activation(mid[:, nsl], ps[:], AF.Identity,
                                         accum_out=sti[:, n:n + 1])
                    sqs = sq_p.tile([128, 512], bf16, tag="sqs")
                    nc.scalar.activation(sqs[:], ps[:], AF.Square,
                                         accum_out=sti[:, 4 + n:5 + n])
            # cols: 8=mu 9=msq 10=musq 11=var 12=lnv 13=rstd 14=nmr
            mu = sti[:, 8:9]
            nc.vector.tensor_reduce(mu, sti[:, 0:4], axis=AX.X, op=ALU.add)
            nc.vector.tensor_scalar_mul(mu, mu, 1.0 / (2.0 * H))
            nc.vector.tensor_reduce(sti[:, 9:10], sti[:, 4:8], axis=AX.X, op=ALU.add)
            nc.vector.tensor_scalar_mul(sti[:, 9:10], sti[:, 9:10], 1.0 / (2.0 * H))
            nc.vector.tensor_tensor(out=sti[:, 10:11], in0=mu, in1=mu, op=ALU.mult)
            nc.vector.tensor_tensor(out=sti[:, 11:12], in0=sti[:, 9:10],
                                    in1=sti[:, 10:11], op=ALU.subtract)
            nc.scalar.activation(sti[:, 12:13], sti[:, 11:12], AF.Ln, bias=eps_s[:])
            nc.scalar.activation(sti[:, 13:14], sti[:, 12:13], AF.Exp, scale=-0.5)
            nc.vector.scalar_tensor_tensor(out=sti[:, 14:15], in0=mu, scalar=-1.0,
                                           in1=sti[:, 13:14], op0=ALU.mult, op1=ALU.mult)

            # gelu((x-mu)*rstd) fused via ACT scale/bias
            midg = midg_p.tile([128, 2 * H], bf16, tag="midg")
            if flags["int_ln"]:
                nc.vector.tensor_scalar(out=midg[:], in0=mid[:], scalar1=sti[:, 8:9],
                                        scalar2=sti[:, 13:14],
                                        op0=ALU.subtract, op1=ALU.mult)
                nc.vector.tensor_tensor(out=midg[:], in0=midg[:], in1=opt_s["g1_rep"][:],
                                        op=ALU.mult)
                nc.vector.tensor_tensor(out=midg[:], in0=midg[:], in1=opt_s["b1ln_rep"][:],
                                        op=ALU.add)
                nc.scalar.activation(midg[:], midg[:], AF.Gelu)
            else:
                nc.scalar.activation(midg[:], mid[:], AF.Gelu,
                                     scale=sti[:, 13:14], bias=sti[:, 14:15])

            # gate (tanh trick: sigmoid(x) = 0.5*(1+tanh(x/2)))
            th = th_p.tile([128, H], bf16, tag="th")
            for n in range(2):
                ps = ps_p.tile([128, 512], f32, tag="ps")
                for c in range(CC):
                    nc.tensor.matmul(ps[:], cat[:, c, tsl], gate_wT_s[:, c, 512 * n:512 * (n + 1)],
                                     start=(c == 0), stop=(c == CC - 1))
                nsl = slice(512 * n, 512 * (n + 1))
                if flags["gate_b"]:
                    gl = tmp_p.tile([128, 512], f32, tag="gl")
                    nc.vector.tensor_tensor(out=gl[:], in0=ps[:],
                                            in1=opt_s["gate_b_rep"][:, nsl], op=ALU.add)
                    nc.scalar.activation(th[:, nsl], gl[:], AF.Tanh, scale=0.5)
                else:
                    nc.scalar.activation(th[:, nsl], ps[:], AF.Tanh, scale=0.5)

            # int2 with inline PE transposes of midg (group-granular)
            psn = [ps_p.tile([128, 512], f32, tag="ps", name=f"psn_{t}_{tch}_{n}")
                   for n in range(2)]
            for g in range(4):
                tp = tp_p.tile([128, 4, 128], bf16, tag="tp")
                for cc in range(4):
                    nc.tensor.transpose(tp[:, cc, :],
                                        midg[:, 128 * (4 * g + cc):128 * (4 * g + cc + 1)],
                                        ident[:])
                mgt = mgt_p.tile([128, 4, 128], bf16, tag="mgt")
                nc.vector.tensor_copy(mgt[:], tp[:])
                for n in range(2):
                    for cc in range(4):
                        nc.tensor.matmul(psn[n][:], mgt[:, cc, :],
                                         w2T_s[:, 4 * g + cc, 512 * n:512 * (n + 1)],
                                         start=(g == 0 and cc == 0),
                                         stop=(g == 3 and cc == 3))

            # residual = x + (tanh+1) * integrated_half ; LN stats inline
            res = res_p.tile([128, H], f32, tag="res")
            stk = st_p.tile([128, 16], f32, tag="stk")
            for n in range(2):
                nsl = slice(512 * n, 512 * (n + 1))
                prod = tmp_p.tile([128, 512], bf16, tag="prod")
                if flags["int_b2"]:
                    t0 = tmp_p.tile([128, 512], f32, tag="t0")
                    nc.vector.tensor_tensor(out=t0[:], in0=psn[n][:],
                                            in1=opt_s["b2h_rep"][:, nsl], op=ALU.add)
                    nc.vector.scalar_tensor_tensor(out=prod[:], in0=th[:, nsl], scalar=1.0,
                                                   in1=t0[:], op0=ALU.add, op1=ALU.mult)
                else:
                    nc.vector.scalar_tensor_tensor(out=prod[:], in0=th[:, nsl], scalar=1.0,
                                                   in1=psn[n][:], op0=ALU.add, op1=ALU.mult)
                nc.vector.scalar_tensor_tensor(out=res[:, nsl], in0=prod[:], scalar=0.0,
                                               in1=xn[:, nsl], op0=ALU.add, op1=ALU.add,
                                               accum_out=stk[:, n:n + 1])
                sqs = sq_p.tile([128, 512], bf16, tag="sqs")
                nc.scalar.activation(sqs[:], res[:, nsl], AF.Square,
                                     accum_out=stk[:, 2 + n:3 + n])
            mu = stk[:, 8:9]
            nc.vector.tensor_reduce(mu, stk[:, 0:2], axis=AX.X, op=ALU.add)
            nc.vector.tensor_scalar_mul(mu, mu, 1.0 / H)
            nc.vector.tensor_reduce(stk[:, 9:10], stk[:, 2:4], axis=AX.X, op=ALU.add)
            nc.vector.tensor_scalar_mul(stk[:, 9:10], stk[:, 9:10], 1.0 / H)
            nc.vector.tensor_tensor(out=stk[:, 10:11], in0=mu, in1=mu, op=ALU.mult)
            nc.vector.tensor_tensor(out=stk[:, 11:12], in0=stk[:, 9:10],
                                    in1=stk[:, 10:11], op=ALU.subtract)
            nc.scalar.activation(stk[:, 12:13], stk[:, 11:12], AF.Ln, bias=eps_s[:])
            nc.scalar.activation(stk[:, 13:14], stk[:, 12:13], AF.Exp, scale=-0.5)

            nc.vector.tensor_scalar(out=res[:], in0=res[:], scalar1=mu,
                                    scalar2=stk[:, 13:14],
                                    op0=ALU.subtract, op1=ALU.mult)
            if flags["ln2"]:
                nc.vector.tensor_tensor(out=res[:], in0=res[:], in1=opt_s["g2_rep"][:],
                                        op=ALU.mult)
                nc.vector.tensor_tensor(out=res[:], in0=res[:], in1=opt_s["b2ln_rep"][:],
                                        op=ALU.add)
            nc.sync.dma_start(out_d[trow + 128 * tch: trow + 128 * (tch + 1), :], res[:])


_BUILD_CACHE = {}


def build_program(flags_key):
    if flags_key in _BUILD_CACHE:
        return _BUILD_CACHE[flags_key]
    flags = dict(flags_key)
    nc = bacc.Bacc("TRN2", target_bir_lowering=False, debug=False)
    with tile.TileContext(nc) as tc:
        with ExitStack() as ctx:
            tc._kernel_ctx = ctx
            _emit(tc, nc, flags)
    nc.compile()
    _BUILD_CACHE[flags_key] = nc
    return nc


def _ln_np(x, g, b, eps=EPS):
    mu = x.mean(-1, keepdims=True)
    var = ((x - mu) ** 2).mean(-1, keepdims=True)
    return (x - mu) / np.sqrt(var + eps) * g + b


def prepare_inputs(query_hidden, mem_keys, selection_scores, qp_w, qp_b,
                   in_proj_w, in_proj_b, out_w, out_b, gate_w, gate_b,
                   int_w1, int_b1, int_ln_g, int_ln_b, int_w2, int_b2,
                   ln1_g, ln1_b, ln2_g, ln2_b, top_k):
    """Host preprocessing: top-k select, mem LN + k/v proj, weight layout."""
    top_k = int(top_k)
    assert top_k == KSEL, f"kernel hardcodes top_k=512, got {top_k}"
    B, S, _ = query_hidden.shape
    assert (B, S) == (8, 2048)

    idx = np.argsort(-selection_scores, kind="stable")[:KSEL]
    mem = mem_keys[idx].astype(np.float32)
    mem_ln = _ln_np(mem, ln1_g, ln1_b)

    wq, wk, wv = np.split(in_proj_w, 3, axis=0)
    bqv, bk, bv = np.split(in_proj_b, 3)
    k = mem_ln @ wk.T + bk            # [KSEL, M]
    v = mem_ln @ wv.T + bv            # [KSEL, M]

    def chunkP(a):  # [K, N] with K=c*128 -> [128, c, N]
        Kd, N = a.shape
        return np.ascontiguousarray(a.reshape(Kd // 128, 128, N).transpose(1, 0, 2))

    def wtile(w):   # torch-style weight [out, in] -> lhsT layout [128, in/128, out]
        return chunkP(np.ascontiguousarray(w.T)).astype(BF)

    def btile(b):   # bias [F] -> [128, F/128, 1] (per-partition, feature-major)
        F = b.shape[0]
        return np.ascontiguousarray(b.reshape(F // 128, 128, 1).transpose(1, 0, 2)).astype(np.float32)

    kT = chunkP(np.ascontiguousarray(k.T)).astype(BF)      # [128, MC, KSEL]
    va = np.zeros((128, KC, NH, HD + 1), np.float32)
    vr = v.reshape(KC, 128, NH, HD)
    va[:, :, :, :HD] = vr.transpose(1, 0, 2, 3)
    va[:, :, :, HD] = 1.0
    va = va.astype(BF)

    xflat = np.ascontiguousarray(query_hidden.reshape(B * S, H)).astype(np.float32)
    xbf_full = xflat.astype(BF)

    flags = {
        "gate_b": bool(np.any(gate_b != 0)),
        "int_b1": bool(np.any(int_b1 != 0)),
        "int_b2": bool(np.any(int_b2 != 0)),
        "int_ln": bool(np.any(int_ln_g != 1) or np.any(int_ln_b != 0)),
        "ln2": bool(np.any(ln2_g != 1) or np.any(ln2_b != 0)),
    }

    shared = {
        "qp_wT": wtile(qp_w),                 # [128, HC, M]
        "qp_b": btile(qp_b),
        "wqT": wtile(wq),
        "bq": btile(bqv),
        "kT": kT,
        "vaug": va,
        "out_wT": wtile(out_w),
        "out_b": btile(out_b),
        "gate_wT": wtile(gate_w),             # [128, CC, H]
        "w1T": wtile(int_w1),                 # [128, CC, 2H]
        "w2T": chunkP(np.ascontiguousarray(int_w2.T) * 0.5).astype(BF),
    }
    if flags["gate_b"]:
        shared["gate_b_rep"] = np.tile(gate_b[None, :], (128, 1)).astype(np.float32)
    if flags["int_b1"]:
        shared["b1_rep"] = np.tile(int_b1[None, :], (128, 1)).astype(np.float32)
    if flags["int_b2"]:
        shared["b2h_rep"] = np.tile(0.5 * int_b2[None, :], (128, 1)).astype(np.float32)
    if flags["int_ln"]:
        shared["g1_rep"] = np.tile(int_ln_g[None, :], (128, 1)).astype(np.float32)
        shared["b1ln_rep"] = np.tile(int_ln_b[None, :], (128, 1)).astype(np.float32)
    if flags["ln2"]:
        shared["g2_rep"] = np.tile(ln2_g[None, :], (128, 1)).astype(np.float32)
        shared["b2ln_rep"] = np.tile(ln2_b[None, :], (128, 1)).astype(np.float32)

    in_maps = []
    for c in range(NCORES):
        rows = slice(c * TPC, (c + 1) * TPC)
        m = dict(shared)
        m["xq"] = xflat[rows]
        m["xbf"] = np.ascontiguousarray(xbf_full[rows].reshape(TPC, HC, 128))
        in_maps.append(m)

    fkey = tuple(sorted(flags.items()))
    return in_maps, fkey, (B, S)


def kernel(**inputs):
    inputs = {k: np.asarray(v) if not np.isscalar(v) else v for k, v in inputs.items()}
    in_maps, fkey, (B, S) = prepare_inputs(**inputs)
    nc = build_program(fkey)
    res = run_bass_kernel_spmd(nc, in_maps, core_ids=list(range(NCORES)))
    out = np.concatenate([res.results[c]["out"] for c in range(NCORES)], axis=0)
    return out.reshape(B, S, H).astype(np.float32)
